# revision 5
# baseline (speedup 1.0000x reference)
"""Deformable Conv1D on 8 Trainium2 NeuronCores (Bass/Tile), axon-tunneled.

Math (reference): out[b,o,l] = sum_{i,k} W[o,i,k] * interp[b,i,l,k] + bias[o]
  interp[b,i,l,k] = lerp of x[b,i,:] at loc = l + k + off[b,l,k], with
  torch-style clamped endpoints (which make any sample with
  loc outside [0, L-1) contribute exactly zero).

Device decomposition per core (core j: batch b=j//2, L-half S=4096*(j%2)):
  Phase 0 (PE+ACT+DVE+Pool): offset conv  off[q,k] (matmul over Cin,K),
    then selector G_k[u,q] = valid(loc) * max(0, 1 - |loc_rel - u|)
    (the lerp-with-clamp is exactly a tent function; clipped samples are
    zero).  Built transposed via per-partition-scalar vector ops, then
    PE-transposed into lhsT layout.
  Phase 1 (PE): Y_k^T[t,o] = sum_i x[b,i,t] * W[o,i,k]   (fp16 matmul)
  Phase 2 (PE): out^T[l,o] = sum_k sum_t G_k[t,l] * Y_k^T[t,o]

Everything is computed on device; the tunnel only moves x (fp16, sharded),
the small weights (replicated, device_put once per call), and the fp16
output back.  The work is split into NCHUNK window-range programs run
back-to-back so the (full-duplex) tunnel overlaps the upload of chunk i+1
with the download of chunk i's output.  All jitted callables are built
once and cached.
"""

import numpy as np

import jax
import jax.numpy as jnp
from jax.sharding import Mesh, PartitionSpec, NamedSharding
from jax.experimental.shard_map import shard_map

import concourse.bacc as bacc
import concourse.bass as bass
import concourse.mybir as mybir
import concourse.tile as tile
import concourse.bass2jax as b2j
from concourse.masks import make_identity

# Problem constants (hardcoded per harness contract).
B, CIN, COUT, L = 4, 256, 256, 8192
K, PAD = 7, 3
NCORE = 8
HALF = L // 2              # 4096 output positions per core
CHUNK = 113                # output positions per window (128-row band covers it)
NWIN = -(-HALF // CHUNK)   # 37
HALO = 4                   # x_pad global col 0 == S - HALO
XPW = 4224                 # padded x width per core (full)
F32 = mybir.dt.float32
F16 = mybir.dt.float16
I32 = mybir.dt.int32
OP = mybir.AluOpType
ACT = mybir.ActivationFunctionType

MASKED_WINS = (0, NWIN - 1)   # only these can have out-of-range samples

# Window-range chunks, pipelined over the duplex tunnel.
WSPLIT = (0, 10, 20, 30, NWIN)
NCHUNK = len(WSPLIT) - 1


def _chunk_geom(c):
    w0, w1 = WSPLIT[c], WSPLIT[c + 1]
    nw = w1 - w0
    width = -(-((nw - 1) * CHUNK + 135) // 16) * 16
    rows = min(CHUNK * w1, HALF) - CHUNK * w0
    return w0, w1, width, rows


_CACHE = {}


def _build_nc(c):
    w0, w1, W, rows_total = _chunk_geom(c)
    nc = bacc.Bacc("TRN2", target_bir_lowering=False, debug=False,
                   num_devices=NCORE)
    x_d = nc.dram_tensor("xp", [2, 128, W], F16, kind="ExternalInput")
    w_d = nc.dram_tensor("wt", [2, K, 128, COUT], F16, kind="ExternalInput")
    ow_d = nc.dram_tensor("ow", [2, K, 128, K], F16, kind="ExternalInput")
    ck_d = nc.dram_tensor("ck", [128, K], F32, kind="ExternalInput")
    bnd_d = nc.dram_tensor("bnd", [128, 4 * K], F32, kind="ExternalInput")
    b_d = nc.dram_tensor("bias", [1, COUT], F32, kind="ExternalInput")
    o_d = nc.dram_tensor("out", [rows_total, COUT], F16, kind="ExternalOutput")

    with tile.TileContext(nc) as tc:
        with (
            tc.tile_pool(name="const", bufs=1) as cpool,
            tc.tile_pool(name="nlp", bufs=2) as nlpool,
            tc.tile_pool(name="dabp", bufs=3) as dabpool,
            tc.tile_pool(name="ttp", bufs=3) as ttpool,
            tc.tile_pool(name="vnp", bufs=4) as vnpool,
            tc.tile_pool(name="gtp", bufs=3) as gtpool,
            tc.tile_pool(name="gsbp", bufs=2) as gsbpool,
            tc.tile_pool(name="ysp", bufs=2) as yspool,
            tc.tile_pool(name="osp", bufs=3) as ospool,
            tc.tile_pool(name="psoff", bufs=1, space="PSUM") as psoff,
            tc.tile_pool(name="psy", bufs=3, space="PSUM") as psy,
            tc.tile_pool(name="pstr", bufs=2, space="PSUM") as pstr,
            tc.tile_pool(name="pso", bufs=2, space="PSUM") as pso,
        ):
            # ---- constants ----
            x_sb = []
            for i in range(2):
                xt = cpool.tile([128, W], F16, tag=f"x{i}")
                nc.sync.dma_start(xt[:], x_d[i])
                x_sb.append(xt)
            w_sb = cpool.tile([128, 2, K, COUT], F16, tag="w")
            nc.sync.dma_start(w_sb[:], w_d.rearrange("i k p o -> p i k o"))
            ow_sb = cpool.tile([128, 2, K, K], F16, tag="ow")
            nc.sync.dma_start(ow_sb[:], ow_d.rearrange("i k p o -> p i k o"))
            ck_sb = cpool.tile([128, K], F32, tag="ck")
            nc.sync.dma_start(ck_sb[:], ck_d[:])
            bnd_sb = cpool.tile([128, 4 * K], F32, tag="bnd")
            nc.sync.dma_start(bnd_sb[:], bnd_d[:])
            bias_row = cpool.tile([1, COUT], F32, tag="br")
            nc.sync.dma_start(bias_row[:], b_d[:])

            ident = cpool.tile([128, 128], F16, tag="id")
            make_identity(nc, ident)

            # iota tiles: u along free (all partitions identical), q down parts
            iota_i = cpool.tile([128, 128], I32, tag="ioti")
            nc.gpsimd.iota(iota_i[:], pattern=[[1, 128]], base=0,
                           channel_multiplier=0)
            iota_f = cpool.tile([128, 128], F32, tag="iotf")
            nc.vector.tensor_copy(iota_f[:], iota_i[:])
            qi_i = cpool.tile([128, 1], I32, tag="qii")
            nc.gpsimd.iota(qi_i[:], pattern=[[1, 1]], base=0,
                           channel_multiplier=1)
            qi_f = cpool.tile([128, 1], F32, tag="qif")
            nc.vector.tensor_copy(qi_f[:], qi_i[:])
            # iota_km[k][u] = u - crow[k]
            iota_km = cpool.tile([128, K, 128], F32, tag="iokm")
            for k in range(K):
                nc.vector.tensor_scalar(iota_km[:, k, :], iota_f[:],
                                        ck_sb[:, k:k + 1], None, OP.subtract)

            # bias tile [128, COUT] via ones-broadcast matmul
            ones_col = cpool.tile([1, 128], F32, tag="oc")
            nc.vector.memset(ones_col[:], 1.0)
            bias_ps = pso.tile([128, COUT], F32, tag="ops")
            nc.tensor.matmul(bias_ps[:], ones_col[:], bias_row[:],
                             start=True, stop=True)
            bias_sb = cpool.tile([128, COUT], F32, tag="bt")
            nc.vector.tensor_copy(bias_sb[:], bias_ps[:])

            # ---- window loop ----
            for ci in range(w0, w1):
                a0 = CHUNK * (ci - w0)   # xp col of band row u=0 (chunk-local)
                # Phase 0a: offset conv -> psum[q,k] (no +q term yet)
                offp = psoff.tile([128, K], F32, tag="offp")
                n = 0
                for i in range(2):
                    for k2 in range(K):
                        lhs = x_sb[i][:, a0 + 1 + k2:a0 + 1 + k2 + 128]
                        nc.tensor.matmul(offp[:], lhs, ow_sb[:, i, k2, :],
                                         start=(n == 0), stop=(n == 13))
                        n += 1
                # nloc[q,k] = -(conv + q)
                nloc = nlpool.tile([128, K], F32, tag="nl")
                nc.vector.tensor_scalar(nloc[:], offp[:], qi_f[:], -1.0,
                                        OP.add, OP.mult)

                # Phase 0b: tent G build, transposed [q,u], then PE transpose
                gsb = gsbpool.tile([128, K, 128], F16, tag="gsb")
                masked = ci in MASKED_WINS
                wi = MASKED_WINS.index(ci) if masked else 0
                for k in range(K):
                    dab = dabpool.tile([128, 128], F32, tag="dab")
                    nc.scalar.activation(dab[:], iota_km[:, k, :], ACT.Abs,
                                         bias=nloc[:, k:k + 1])
                    tt = ttpool.tile([128, 128], F32, tag="tt")
                    nc.vector.tensor_scalar(tt[:], dab[:], 1.0, 0.0,
                                            OP.subtract, OP.min)
                    gt = gtpool.tile([128, 128], F16, tag="gt")
                    if masked:
                        c0 = 2 * K * wi + 2 * k
                        v1 = vnpool.tile([128, 1], F32, tag="v1")
                        nc.vector.tensor_scalar(
                            v1[:], nloc[:, k:k + 1], bnd_sb[:, c0:c0 + 1],
                            -1.0, OP.is_le, OP.mult)
                        vn = vnpool.tile([128, 1], F32, tag="vn")
                        nc.vector.scalar_tensor_tensor(
                            vn[:], nloc[:, k:k + 1], bnd_sb[:, c0 + 1:c0 + 2],
                            v1[:], OP.is_gt, OP.mult)
                        nc.gpsimd.tensor_scalar(gt[:], tt[:], vn[:], None,
                                                OP.mult)
                    else:
                        nc.gpsimd.tensor_scalar(gt[:], tt[:], -1.0, None,
                                                OP.mult)
                    trp = pstr.tile([128, 128], F16, tag="trp")
                    nc.tensor.transpose(trp[:], gt[:], ident[:])
                    nc.vector.tensor_copy(gsb[:, k, :], trp[:])

                # Phase 1: Y_k^T[t,o]
                ys = yspool.tile([128, K, COUT], F16, tag="ys")
                for k in range(K):
                    yp = psy.tile([128, COUT], F32, tag="yps")
                    for i in range(2):
                        lhs = x_sb[i][:, a0:a0 + 128]
                        nc.tensor.matmul(yp[:], lhs, w_sb[:, i, k, :],
                                         start=(i == 0), stop=(i == 1))
                    nc.scalar.copy(ys[:, k, :], yp[:])

                # Phase 2: out^T[q,o] = sum_k G_k^T @ Y_k^T
                ops = pso.tile([128, COUT], F32, tag="ops")
                for k in range(K):
                    nc.tensor.matmul(ops[:], gsb[:, k, :], ys[:, k, :],
                                     start=(k == 0), stop=(k == K - 1))
                osb = ospool.tile([128, COUT], F16, tag="o")
                nc.vector.tensor_add(osb[:], ops[:], bias_sb[:])
                r0 = CHUNK * (ci - w0)
                rows = min(CHUNK, rows_total - r0)
                nc.sync.dma_start(o_d[r0:r0 + rows, :], osb[:rows, :])

    nc.finalize()
    return nc


def _make_fn(nc):
    """One cached jitted PJRT callable for a Bass program (axon path)."""
    partition_name = (nc.partition_id_tensor.name
                      if nc.partition_id_tensor else None)
    in_names, out_names, out_avals = [], [], []
    for alloc in nc.m.functions[0].allocations:
        if not isinstance(alloc, mybir.MemoryLocationSet):
            continue
        name = alloc.memorylocations[0].name
        if alloc.kind == "ExternalInput":
            if name != partition_name:
                in_names.append(name)
        elif alloc.kind == "ExternalOutput":
            out_names.append(name)
            out_avals.append(jax.core.ShapedArray(
                tuple(alloc.tensor_shape), mybir.dt.np(alloc.dtype)))
    n_params = len(in_names)
    n_outs = len(out_names)
    all_in_names = list(in_names) + list(out_names)
    if partition_name is not None:
        all_in_names.append(partition_name)
    donate = tuple(range(n_params, n_params + n_outs))

    def _body(*args):
        operands = list(args)
        if partition_name is not None:
            operands.append(b2j.partition_id_tensor())
        outs = b2j._bass_exec_p.bind(
            *operands,
            out_avals=tuple(out_avals),
            in_names=tuple(all_in_names),
            out_names=tuple(out_names),
            lowering_input_output_aliases=(),
            sim_require_finite=True,
            sim_require_nnan=True,
            nc=nc,
        )
        return tuple(outs)

    devices = jax.devices()[:NCORE]
    mesh = Mesh(np.asarray(devices), ("core",))
    sharded_names = {"xp", "bnd"}       # per-core inputs; rest replicated
    in_specs = tuple(
        PartitionSpec("core") if n in sharded_names else PartitionSpec()
        for n in in_names
    ) + (PartitionSpec("core"),) * n_outs
    out_specs = (PartitionSpec("core"),) * n_outs
    fn = jax.jit(
        shard_map(_body, mesh=mesh, in_specs=in_specs,
                  out_specs=out_specs, check_rep=False),
        donate_argnums=donate, keep_unused=True,
    )
    zsh = NamedSharding(mesh, PartitionSpec("core"))
    zshapes = [(NCORE * a.shape[0], *a.shape[1:]) for a in out_avals]
    zdtypes = [a.dtype for a in out_avals]
    zfn = jax.jit(
        lambda: tuple(jnp.zeros(s, d) for s, d in zip(zshapes, zdtypes)),
        out_shardings=(zsh,) * n_outs,
    )
    return fn, zfn, in_names


def _get_runner():
    if "runner" in _CACHE:
        return _CACHE["runner"]
    b2j.install_neuronx_cc_hook()
    chunks = [_make_fn(_build_nc(c)) for c in range(NCHUNK)]
    mesh = Mesh(np.asarray(jax.devices()[:NCORE]), ("core",))
    shardings = {
        "xp": NamedSharding(mesh, PartitionSpec("core")),
        "bnd": NamedSharding(mesh, PartitionSpec("core")),
        "wt": NamedSharding(mesh, PartitionSpec()),
        "ow": NamedSharding(mesh, PartitionSpec()),
        "ck": NamedSharding(mesh, PartitionSpec()),
        "bias": NamedSharding(mesh, PartitionSpec()),
    }
    _CACHE["runner"] = (chunks, shardings)
    return _CACHE["runner"]


def _host_prep(x, weight, bias, offset_w, offset_b):
    """Pure data movement: slice/cast per-core inputs. No compute."""
    x = np.asarray(x, np.float32)
    weight = np.asarray(weight, np.float32)
    bias = np.asarray(bias, np.float32)
    offset_w = np.asarray(offset_w, np.float32)
    offset_b = np.asarray(offset_b, np.float32)

    xq = x.astype(np.float16)
    xp = np.zeros((NCORE, 2, 128, XPW), np.float16)
    bnd = np.empty((NCORE, 128, 4 * K), np.float32)
    crow = np.arange(K, dtype=np.float32) + HALO + offset_b
    for core in range(NCORE):
        b, half = divmod(core, 2)
        S = HALF * half
        lo, hi = S - HALO, S - HALO + XPW
        cl, ch = max(0, lo), min(L, hi)
        xp[core, :, :, cl - lo:ch - lo] = (
            xq[b, :, cl:ch].reshape(2, 128, ch - cl))
        for wi, win in enumerate(MASKED_WINS):
            band0 = S + win * CHUNK - HALO
            A = band0 + crow                       # valid: nloc <= A
            Bv = A - (L - 1)                       # and nloc > B
            bnd[core, :, 2 * K * wi + 0:2 * K * wi + 2 * K:2] = A
            bnd[core, :, 2 * K * wi + 1:2 * K * wi + 2 * K:2] = Bv

    wt = np.ascontiguousarray(
        weight.reshape(COUT, 2, 128, K).transpose(1, 3, 2, 0)).astype(np.float16)
    ow = np.ascontiguousarray(
        offset_w.transpose(1, 2, 0).reshape(2, 128, K, K).transpose(0, 2, 1, 3)
    ).astype(np.float16)
    ck = np.tile(crow, (128, 1)).astype(np.float32)

    xp_chunks = []
    for c in range(NCHUNK):
        w0, _, Wc, _ = _chunk_geom(c)
        x0 = CHUNK * w0
        xp_chunks.append(np.ascontiguousarray(
            xp[:, :, :, x0:x0 + Wc]).reshape(NCORE * 2, 128, Wc))

    return {
        "xp_chunks": xp_chunks,
        "bnd": bnd.reshape(NCORE * 128, 4 * K),
        "wt": wt,
        "ow": ow,
        "ck": ck,
        "bias": bias.reshape(1, COUT),
    }


def _device_call(prepped):
    """The timed region: numpy in -> numpy out.  Chunks are dispatched
    back-to-back; worker threads fetch each chunk's output while the main
    thread uploads the next chunk's input (the tunnel is full-duplex, but
    copy_to_host_async does not overlap on this backend — blocking
    np.asarray calls in threads do)."""
    from concurrent.futures import ThreadPoolExecutor
    chunks, shardings = _get_runner()
    if "pool" not in _CACHE:
        _CACHE["pool"] = ThreadPoolExecutor(max_workers=2)
    pool = _CACHE["pool"]
    shared = {
        n: jax.device_put(prepped[n], shardings[n])
        for n in ("wt", "ow", "ck", "bnd", "bias")
    }
    futs = []
    for c, (fn, zfn, in_names) in enumerate(chunks):
        zeros = zfn()
        args = [prepped["xp_chunks"][c] if n == "xp" else shared[n]
                for n in in_names]
        out = fn(*args, *zeros)[0]
        futs.append(pool.submit(np.asarray, out))
    return [f.result() for f in futs]


def _assemble(out_list):
    out = np.empty((B, COUT, L), np.float32)
    per_core = [np.concatenate(
        [o.reshape(NCORE, -1, COUT)[c] for o in out_list], axis=0)
        for c in range(NCORE)]
    for core in range(NCORE):
        b, half = divmod(core, 2)
        S = HALF * half
        out[b, :, S:S + HALF] = per_core[core].T.astype(np.float32)
    return out


def kernel(x, weight, bias, offset_w, offset_b):
    prepped = _host_prep(x, weight, bias, offset_w, offset_b)
    return _assemble(_device_call(prepped))


def kernel_timed(inputs, repeats=3):
    """Dev helper: returns (out, wall_times_s per device roundtrip)."""
    import time
    prepped = _host_prep(**inputs)
    _get_runner()
    times, og = [], None
    for _ in range(repeats):
        t0 = time.time()
        og = _device_call(prepped)
        times.append(time.time() - t0)
    return _assemble(og), times
